# revision 47
# baseline (speedup 1.0000x reference)
"""Trainium2 Bass kernel for nn_DCT_base_Rec_Module (topk patch selection).

Math: band_filter(0, 64, 32) is all-ones and D (orthonormal DCT-II) satisfies
D^T D = I, so the reference's iDCT output y equals the raw input patches
exactly (up to fp rounding).  The device therefore only computes the per-patch
grade
    grade[l] = sum_{c,f1,f2} log(|S_l,c,f1,f2| + 1) * W[c,f1,f2],
    S = D X D^T  (per 32x32 patch, stride 16 -> L = 127*127),
sharded over the 127 patch rows across 8 cores; the host argsorts the 16129
grades and slices the 4 winning patches straight out of the fp32 input.

Device pipeline per core (16 patch rows, fp16 storage / fp32 PSUM),
software-pipelined so PE / DVE / ACT all stay ~80%+ busy:
  stage 1  (PE):  x loaded once (272 unique rows as two 128-row chunks + a
                  16-row tail); three matmuls with half-height DCT operators
                  accumulate V^T [128 cols, (16 win, 32 f1)] per column-tile.
                  Jobs flow through a queue interleaved one channel ahead of
                  stage 2 so the in-order PE queue never idles.
  copy     (~3/4 ACT, 1/4 DVE): V^T psum -> sbuf fp16.  ACT-routed copies
                  have a full channel of slack and fill ACT's gaps between
                  Ln ops.
  stage 2  (PE):  lhsT = shifted-blockdiag D^T (even / odd+tail windows),
                  rhs = V^T tiles          ->  S [ (4w,32 f2), (16 win,32 f1) ]
  abs      (DVE bitwise-and, always): psum -> sbuf fp32; keeping abs off ACT
                  keeps the stage-2 psum recycle chain off the Ln critical
                  path.
  log      (ACT Ln, bias=1)                 sbuf -> sbuf fp16
  reduce   (PE):  f1-sliced accumulating matmuls with W slices -> grades,
                  emitted two chunks late so the in-order PE queue never
                  stalls waiting for Ln; grades ship out in two DMA halves.
Input DMAs are column-quartered in first-use order so compute starts ~4us in;
a dummy Ln pulls the ACT table load into the DMA window and dummy matmuls
warm the PE clock.
"""

import numpy as np

WS = 32
STRIDE = 16
H = 2048
NCORES = 8
NT = 16            # 128-col V^T tiles per row (2048/128)
ROWS_PER_CORE = 16  # patch rows per core (core 7: 15 valid)
SLAB_ROWS = 272     # 16*16 + 16 halo

# fraction of the 48 stage-1 copies that run on ACT (rest on DVE); tuned via sim
CP_ON_ACT = 35 / 48


def _dct_mat():
    i = np.arange(WS)[:, None].astype(np.float64)
    j = np.arange(WS)[None, :].astype(np.float64)
    m = np.sqrt(2.0 / WS) * np.cos((j + 0.5) * np.pi * i / WS)
    m[0, :] = np.sqrt(1.0 / WS)
    return m.astype(np.float32)


def _s1_consts_np():
    """Half-height DCT operators for dedup stage 1: chunk A (rows 0:128,
    strips 0..7) covers windows w0..w6 + first half of w7; chunk B (rows
    128:256) covers second half of w7, w8..w14, first half of w15; chunk C
    (rows 256:272) covers the second half of w15.  ps[col, 32*w + f1]."""
    D = _dct_mat()
    s1a = np.zeros((128, 256), np.float32)
    for w in range(8):
        for k in range(16):
            s1a[16 * w + k, 32 * w:32 * w + 32] = D[:, k]
            if w < 7:
                s1a[16 * (w + 1) + k, 32 * w:32 * w + 32] = D[:, 16 + k]
    s1b = np.zeros((128, 288), np.float32)
    for k in range(16):
        s1b[k, 0:32] = D[:, 16 + k]
        s1b[16 * 7 + k, 256:288] = D[:, k]
    for m in range(7):
        for k in range(16):
            s1b[16 * m + k, 32 * (m + 1):32 * (m + 2)] = D[:, k]
            s1b[16 * (m + 1) + k, 32 * (m + 1):32 * (m + 2)] = D[:, 16 + k]
    s1c = np.ascontiguousarray(D[:, 16:32].T)
    return (s1a.astype(np.float16), s1b.astype(np.float16),
            s1c.astype(np.float16))


def _consts_np():
    D = _dct_mat()
    Dt = D.T.copy()  # [jc, f2] = D[f2, jc]
    bde = np.zeros((128, 128), np.float32)
    for w in range(4):
        bde[32 * w:32 * w + 32, 32 * w:32 * w + 32] = Dt
    l2o = np.zeros((128, 128), np.float32)
    for w in range(4):
        r0 = 16 + 32 * w
        r1 = min(r0 + 32, 128)
        l2o[r0:r1, 32 * w:32 * w + 32] = Dt[: r1 - r0, :]
    l2t = np.zeros((128, 128), np.float32)
    l2t[0:16, 96:128] = Dt[16:32, :]
    return (bde.astype(np.float16), l2o.astype(np.float16),
            l2t.astype(np.float16))


def _wred_np(W):
    # wred[c, f1, (32*w + f2), w'] = delta_{w,w'} * W[c, f1, f2]
    out = np.zeros((3, 32, 128, 4), np.float32)
    for c in range(3):
        for f1 in range(32):
            for w in range(4):
                out[c, f1, 32 * w:32 * w + 32, w] = W[c, f1, :]
    return out.astype(np.float16)


_BUILt = {}


def _build_program():
    if "nc" in _BUILt:
        return _BUILt["nc"]
    from contextlib import ExitStack
    import concourse.bass as bass
    import concourse.tile as tile
    from concourse import bacc, mybir

    f16 = mybir.dt.float16
    f32 = mybir.dt.float32

    nc = bacc.Bacc("TRN2", target_bir_lowering=False, debug=False)

    xs_d = nc.dram_tensor("xs", [3, 2, 128, H], f16, kind="ExternalInput")
    xsc_d = nc.dram_tensor("xsc", [3, 16, H], f16, kind="ExternalInput")
    # one blob: [0:576] s1abc | [576:704] bde | [704:832] l2o | [832:960] l2t
    # | [960:1344] wred pre-permuted to [p=(32w+f2), (c*32+f1)*4 + w']
    cst_d = nc.dram_tensor("cst", [128, 1344], f16, kind="ExternalInput")
    gr_d = nc.dram_tensor("grades", [4, 512], f32, kind="ExternalOutput")

    with tile.TileContext(nc) as tc, ExitStack() as ctx:
        const = ctx.enter_context(tc.tile_pool(name="const", bufs=1))
        xsp = ctx.enter_context(tc.tile_pool(name="xsp", bufs=1))
        vtp = ctx.enter_context(tc.tile_pool(name="vtp", bufs=20))
        sap = ctx.enter_context(tc.tile_pool(name="sap", bufs=3))
        tp = ctx.enter_context(tc.tile_pool(name="tp", bufs=3))
        s1pp = ctx.enter_context(tc.tile_pool(name="s1pp", bufs=3, space="PSUM"))
        s2pp = ctx.enter_context(tc.tile_pool(name="s2pp", bufs=2, space="PSUM"))
        grpp = ctx.enter_context(tc.tile_pool(name="grpp", bufs=1, space="PSUM"))

        cst_s = const.tile([128, 1344], f16, tag="cst")
        s1abc_s = cst_s[:, 0:576]
        bde_s = cst_s[:, 576:704]
        l2o_s = cst_s[:, 704:832]
        l2t_s = cst_s[:, 832:960]
        wred_s = cst_s[:, 960:1344]


        # ---- chunked input DMAs: channel 0 lands in column-quarters so its
        # first stage-1 tiles can start ~3us in; c1/c2 per (c, group). ----
        xs = []
        xsc = []
        for c in range(3):
            xst = xsp.tile([128, 2 * H], f16, tag=f"xs{c}")
            xs.append(xst)
            xsct = xsp.tile([16, H], f16, tag=f"xsc{c}")
            xsc.append(xsct)
        HQ = H // 4

        def xq_dma(c, q):
            # one DMA per column-quarter covering both 128-row chunks
            nc.sync.dma_start(
                bass.AP(xs[c].tensor, q * HQ, [[2 * H, 128], [H, 2], [1, HQ]]),
                bass.AP(xs_d, c * 2 * 128 * H + q * HQ,
                        [[H, 128], [128 * H, 2], [1, HQ]]),
            )

        def xsc_dma(c):
            nc.sync.dma_start(
                bass.AP(xsc[c].tensor, 0, [[H, 16], [1, H]]),
                bass.AP(xsc_d, c * 16 * H, [[H, 16], [1, H]]),
            )

        # in order of first use: stage-1 consts + c0 cols 0:1024, stage-2
        # consts, rest of c0, then c1/c2 (all column-quartered so the stage-1
        # pump never convoys the in-order PE queue on a late DMA)
        nc.sync.dma_start(s1a_s[:], s1a_d.ap())
        nc.sync.dma_start(s1b_s[:], s1b_d.ap())
        nc.sync.dma_start(s1c_s[:], s1c_d.ap())
        xsc_dma(0)
        xq_dma(0, 0)
        nc.sync.dma_start(bde_s[:], bde_d.ap())
        xq_dma(0, 1)
        nc.sync.dma_start(l2o_s[:], l2o_d.ap())
        nc.sync.dma_start(l2t_s[:], l2t_d.ap())
        xq_dma(0, 2)
        xq_dma(0, 3)
        # wred sbuf layout: [p=(32w+f2), (c*32+f1)*4 + w']
        nc.sync.dma_start(
            bass.AP(wred_s.tensor, 0, [[384, 128], [4, 96], [1, 4]]),
            bass.AP(wred_d, 0, [[4, 128], [128 * 4, 96], [1, 4]]),
        )
        xsc_dma(1)
        xsc_dma(2)
        for c in (1, 2):
            for q in range(4):
                xq_dma(c, q)

        gp = grpp.tile([4, 512], f32, tag="grp")
        # start=True clears has_written for the WHOLE psum bank, so the four
        # (b, par) accumulation groups sharing this bank cannot each open with
        # start=True (each later open discards earlier groups' partial sums).
        # Instead zero the bank once (has_written set by writing zeros) and
        # accumulate every real reduce matmul with start=False.
        z4 = const.tile([128, 4], f16, tag="z4")
        nc.vector.memset(z4[:], 0)
        # dummy Ln at t=0: pulls the natural_log table load (which covers Ln,
        # Abs and Copy) into the DMA-wait window instead of the first chunk
        lnwarm = const.tile([128, 4], f16, tag="lnwarm")
        nc.scalar.activation(lnwarm[:], z4[:],
                             mybir.ActivationFunctionType.Ln, bias=1.0)
        wu = s1pp.tile([128, 512], f32, tag="s1")
        # PE warmup: ~3us of dummy matmuls into the grades bank before it is
        # zeroed, so the first real stage-1 matmul runs at full clock
        for _ in range(int(os.environ.get("K_WARM", "12"))):
            nc.tensor.matmul(gp[:, 0:128], z4[:], bde_s[:],
                             start=True, stop=False, skip_group_check=True)
        for sl in range(4):
            nc.tensor.matmul(gp[:, sl * 128:(sl + 1) * 128], z4[:], bde_s[:],
                             start=True, stop=False, skip_group_check=True)

        # vt tiles per (channel, t); stage-1 jobs run from a queue interleaved
        # into the stage-2 stream so the PE never sits on a long serial S1 run
        vts = [[None] * NT for _ in range(3)]

        cp_state = [0.99, 0]

        def s1_emit(c, t, force_eng=None):
            """stage 1 for tile t of channel c + psum->sbuf copy.

            The copy mostly runs on ACT (it has slack of a full channel and
            fills ACT's gaps between Ln ops); the rest go to DVE.  All abs ops
            live on DVE so the stage-2 psum recycle chain never waits on ACT.
            """
            ps = s1pp.tile([128, 512], f32, tag="s1")
            nc.tensor.matmul(ps[:, 0:256],
                             xs[c][:, 128 * t:128 * (t + 1)],
                             s1abc_s[:, 0:256],
                             start=True, stop=False, skip_group_check=True)
            nc.tensor.matmul(ps[:, 224:512],
                             xs[c][:, H + 128 * t:H + 128 * (t + 1)],
                             s1abc_s[:, 256:544],
                             start=False, stop=False, skip_group_check=True)
            nc.tensor.matmul(ps[:, 480:512],
                             xsc[c][:, 128 * t:128 * (t + 1)],
                             s1abc_s[0:16, 544:576],
                             start=False, stop=True, skip_group_check=True)
            vt = vtp.tile([128, 512], f16, tag="vt")
            if force_eng is None:
                cp_state[0] += CP_ON_ACT
                if cp_state[0] >= 1.0:
                    cp_state[0] -= 1.0
                    force_eng = "act"
                else:
                    force_eng = "dve"
            if force_eng == "act":
                nc.scalar.copy(vt[:], ps[:])
            else:
                nc.vector.tensor_copy(vt[:], ps[:])
            vts[c][t] = vt

        gr_sb = const.tile([4, 512], f32, tag="gr")

        def gr_out(c0, nc_):
            """copy + DMA a finished column range of the grades psum."""
            nc.vector.tensor_copy(gr_sb[:, c0:c0 + nc_], gp[:, c0:c0 + nc_])
            nc.sync.dma_start(
                bass.AP(gr_d, c0, [[512, 4], [1, nc_]]),
                gr_sb[:, c0:c0 + nc_])

        def reduce_emit(c, b, tt0, ntt, tb):
            """accumulate tiles [tt0, tt0+ntt) of (c, b)'s tb into grades."""
            for par in range(2):
                for f1 in range(32):
                    nc.tensor.matmul(
                        gp[:, (b * 2 + par) * 128 + 16 * tt0:
                           (b * 2 + par) * 128 + 16 * (tt0 + ntt)],
                        wred_s[:, (c * 32 + f1) * 4:(c * 32 + f1) * 4 + 4],
                        bass.AP(tb.tensor, par * 512 + f1 + 1024 * tt0,
                                [[8 * 1024, 128], [1024, ntt], [32, 16]]),
                        start=False,
                        stop=(c == 2 and f1 == 31),
                        skip_group_check=True,
                    )
            if c == 2 and b == 0 and tt0 + ntt == 8:
                gr_out(0, 256)


        s1_jobs = [(c, t) for c in range(3) for t in range(NT)]
        _pump_state = [0]

        def s1_pump(n, force_eng=None):
            for _ in range(n):
                if _pump_state[0] < len(s1_jobs):
                    s1_emit(*s1_jobs[_pump_state[0]], force_eng=force_eng)
                    _pump_state[0] += 1

        # prologue: first 4 stage-1 tiles of channel 0 (exactly the first
        # column-quarter DMA; overlaps the rest of the input DMA and warms up
        # the PE); the rest flow through the job queue
        s1_pump(4)

        # chunk list: (c, b, tt0, ntt); small chunks at the start (fill the
        # ACT pipeline sooner) and at the end (shorter serial tail)
        chunks = []
        for c in range(3):
            for b in range(2):
                if c == 0 and b == 0:
                    parts = [(0, 2), (2, 2), (4, 4)]
                elif c == 2 and b == 1:
                    parts = [(0, 4), (4, 2), (6, 1), (7, 1)]
                else:
                    parts = ([(0, 4), (4, 4)]
                             if os.environ.get("K_MID", "44") == "44"
                             else [(0, 8)])
                for tt0, ntt in parts:
                    chunks.append((c, b, tt0, ntt))

        pending_red = []
        tb = None
        tb_key = None
        for (c, b, tt0, ntt) in chunks:
            if tb_key != (c, b):
                tb = tp.tile([128, 8 * 1024], f16, tag="tb")
                tb_key = (c, b)
            sa = sap.tile([128, 1024 * ntt], f32, tag=f"sa{ntt}")
            for tt in range(tt0, tt0 + ntt):
                t = 8 * b + tt
                ps = s2pp.tile([128, 1024], f32, tag="s2")
                nc.tensor.matmul(ps[:, 0:512], bde_s[:],
                                 vts[c][t][:], start=True, stop=True)
                last = (t == NT - 1)
                nc.tensor.matmul(ps[:, 512:1024], l2o_s[:],
                                 vts[c][t][:], start=True, stop=last)
                if not last:
                    nc.tensor.matmul(ps[:, 512:1024], l2t_s[:],
                                     vts[c][t + 1][:], start=False,
                                     stop=True)
                dst = sa[:, 1024 * (tt - tt0):1024 * (tt - tt0 + 1)]
                nc.vector.tensor_scalar(
                    dst.bitcast(mybir.dt.int32),
                    ps[:].bitcast(mybir.dt.int32),
                    0x7FFFFFFF, None,
                    mybir.AluOpType.bitwise_and)
                # keep the stage-1 pipeline fed (after the abs so the
                # DVE runs abs before the next copy)
                s1_pump(1)
            nc.scalar.activation(
                tb[:, 1024 * tt0:1024 * (tt0 + ntt)],
                sa[:],
                mybir.ActivationFunctionType.Ln,
                bias=1.0,
            )
            pending_red.append((c, b, tt0, ntt, tb))
            # emit the reduce lagged by one chunk so its Ln is long
            # done by the time the in-order PE queue reaches it
            if len(pending_red) > 1:
                reduce_emit(*pending_red.pop(0))
        # keep the PE clock warm through the final Ln so the last reduces run
        # at full speed
        for _ in range(int(os.environ.get("K_WARM2", "12"))):
            nc.tensor.matmul(wu[0:4, 0:128], z4[:], bde_s[:],
                             start=True, stop=False, skip_group_check=True)
        while pending_red:
            reduce_emit(*pending_red.pop(0))
        gr_out(256, 256)

    nc.compile()
    _BUILt["nc"] = nc
    return nc


def _make_in_maps(x, W):
    bde, l2o, l2t = _consts_np()
    s1a, s1b, s1c = _s1_consts_np()
    wred = _wred_np(W[0].astype(np.float32))
    x16 = x.astype(np.float16)
    in_maps = []
    for k in range(NCORES):
        r0 = 256 * k
        slab = np.zeros((3, SLAB_ROWS, H), np.float16)
        rows = min(SLAB_ROWS, H - r0)
        slab[:, :rows, :] = x16[:, r0:r0 + rows, :]
        xs = np.empty((3, 2, 128, H), np.float16)
        xs[:, 0] = slab[:, 0:128]
        xs[:, 1] = slab[:, 128:256]
        xsc = np.ascontiguousarray(slab[:, 256:272])
        cst = np.zeros((128, 1344), np.float16)
        cst[:, 0:256] = s1a
        cst[:, 256:544] = s1b
        cst[0:16, 544:576] = s1c
        cst[:, 576:704] = bde
        cst[:, 704:832] = l2o
        cst[:, 832:960] = l2t
        # wred pre-permuted: [p=(32w+f2), (c*32+f1)*4 + w']
        cst[:, 960:1344] = wred.transpose(2, 0, 1, 3).reshape(128, 384)
        in_maps.append({"xs": xs, "xsc": xsc, "cst": cst})
    return in_maps


def _decode_grades(res):
    """res: list per core of {'grades': [4,512] f32} -> full grades [16129]."""
    full = np.full(127 * 127, np.nan, np.float32)
    for k in range(NCORES):
        g = res[k]["grades"]  # [w', 512]
        for b in range(2):
            for par in range(2):
                blk = g[:, (b * 2 + par) * 128:(b * 2 + par + 1) * 128]
                for wq in range(4):
                    for n in range(128):
                        tt, kk = divmod(n, 16)
                        t = 8 * b + tt
                        jw = 8 * t + 2 * wq + par
                        i_loc = kk
                        i_glob = ROWS_PER_CORE * k + i_loc
                        if i_glob <= 126 and jw <= 126:
                            full[127 * i_glob + jw] = blk[wq, n]
    assert not np.isnan(full).any()
    return full


LAST_EXEC_NS = None


def kernel(x, W):
    global LAST_EXEC_NS
    x = np.asarray(x)
    W = np.asarray(W)
    nc = _build_program()
    from concourse.bass_utils import run_bass_kernel_spmd
    in_maps = _make_in_maps(x, W)
    out = run_bass_kernel_spmd(nc, in_maps, core_ids=list(range(NCORES)))
    LAST_EXEC_NS = out.exec_time_ns
    grades = _decode_grades(out.results)
    idx = np.argsort(grades, kind="stable")

    def patch(l):
        i, j = divmod(int(l), 127)
        return x[:, 16 * i:16 * i + 32, 16 * j:16 * j + 32].astype(np.float32)

    return (patch(idx[0]), patch(idx[-1]), patch(idx[1]), patch(idx[-2]))
